# revision 21
# baseline (speedup 1.0000x reference)
"""Trainium2 Bass kernel for ViT attention with LSA (per-head scale, masked diag).

Full inputs in, full outputs out. Sharding: data-parallel over batch across
8 NeuronCores (4 batches each). No collectives.

Per-core pipeline (per batch; tokens host-padded 577 -> 640 with zeros):
  xT [768, 640]   via hardware DMA-transpose of host-cast bf16 x
  qT,kT = W_{q,k}^T @ x^T       (bf16 matmul -> bf16 tiles, feature-major,
                                 LSA scale folded into Wq on host)
  v     = x @ W_v               (token-major + ones column -> v_ext, bf16;
                                 ones zeroed on pad rows)
  dotsT[j,i] = kT^T qT          (bf16 matmul, PSUM f32, per head)
  A = exp(dotsT)                (ACT, PSUM->SBUF bf16; no max-sub needed)
  A[diag block] *= (1 - I_128)  (DVE, only the diagonal 128-window per j-tile)
  out_ext[65, i] = v_ext^T A    (bf16 matmul; row 64 = softmax denominator)
  attn_out = out_ext[0:64] * recip(denominator)       (DVE -> f32r)
  out = attn_out^T @ W_out + b  (fp32r matmul, token-major, contiguous DMA)

Batches are software-pipelined: batch b+1's projections interleave into
batch b's attention stream to keep the PE dense (HAM warm).
"""

import numpy as np

B, N, DIM = 32, 577, 768
H, DH = 12, 64
NCORES = 8
BPC = B // NCORES          # batches per core
T = N                      # real tokens per batch
NP = 580                   # padded i-width (2 chunks of 290)
CH = 290                   # i-chunk size
XW = 640                   # DMA-transpose width (needs %128 free dim)
TT = [(i * 128, min(128, T - i * 128)) for i in range((T + 127) // 128)]  # j/t tiles

_cache = {}


def _build_nc():
    import concourse.bass as bass
    import concourse.tile as tile
    from concourse import bacc, mybir

    f32 = mybir.dt.float32
    f32r = mybir.dt.float32r
    bf16 = mybir.dt.bfloat16
    AF = mybir.ActivationFunctionType
    OP = mybir.AluOpType

    nc = bacc.Bacc("TRN2", target_bir_lowering=False, debug=False)

    x_d = nc.dram_tensor("x", [BPC, XW, DIM], bf16, kind="ExternalInput").ap()
    wqkv_d = nc.dram_tensor("wqkv", [DIM, 3 * DIM], bf16, kind="ExternalInput").ap()
    wout_d = nc.dram_tensor("wout", [DIM, DIM], f32, kind="ExternalInput").ap()
    bout_d = nc.dram_tensor("bout", [DIM], f32, kind="ExternalInput").ap()
    mask_d = nc.dram_tensor("mask", [128, 128], bf16, kind="ExternalInput").ap()
    out_d = nc.dram_tensor("out", [BPC, N, DIM], f32, kind="ExternalOutput").ap()

    with tile.TileContext(nc) as tc:
        with (
            tc.tile_pool(name="weights", bufs=1) as wp,
            tc.tile_pool(name="xt", bufs=2) as xtp,
            tc.tile_pool(name="qkt", bufs=2) as qkp,
            tc.tile_pool(name="vext", bufs=2) as vp,
            tc.tile_pool(name="aout", bufs=4) as aop,
            tc.tile_pool(name="apool", bufs=5) as apl,
            tc.tile_pool(name="small", bufs=2) as smp,
            tc.tile_pool(name="ostage", bufs=2) as osp,
            tc.tile_pool(name="pmisc", bufs=2, space="PSUM") as pmp,
            tc.tile_pool(name="pdots", bufs=1, space="PSUM") as pdp,
            tc.tile_pool(name="poext", bufs=1, space="PSUM") as pop,
        ):
            # ---- static tiles ----
            wqkv = wp.tile([128, 6, 3 * DIM], bf16)
            for kc in range(6):
                nc.sync.dma_start(
                    out=wqkv[:, kc, :], in_=wqkv_d[kc * 128 : (kc + 1) * 128, :]
                )
            wout = wp.tile([128, 6, DIM], f32r)
            for kc in range(6):
                nc.gpsimd.dma_start(
                    out=wout[:, kc, :], in_=wout_d[kc * 128 : (kc + 1) * 128, :]
                )
            mask = wp.tile([128, 128], bf16)
            nc.sync.dma_start(out=mask, in_=mask_d)
            b_bc = wp.tile([128, DIM], f32)
            bout_bcast = bass.AP(
                tensor=bout_d.tensor, offset=bout_d.offset, ap=[[0, 128], [1, DIM]]
            )
            nc.sync.dma_start(out=b_bc, in_=bout_bcast)

            zmaster = wp.tile([128, 12], f32)
            nc.vector.memset(zmaster[:, :], 0.0)
            omaster = wp.tile([128, 12], f32)
            nc.vector.memset(omaster[:, :], 1.0)

            state = {}  # b -> dict(xT=, qkT=, vext=)

            def prep_units(b):
                """Thunks for batch b's load/projection work."""
                if b >= BPC:
                    return []
                st = {}
                state[b] = st
                units = []

                def xt_unit():
                    def run():
                        st["xT"] = xtp.tile([128, 6, XW], bf16, tag="xT", name="xT")
                        for dc in range(6):
                            nc.sync.dma_start_transpose(
                                out=st["xT"][:, dc, :],
                                in_=x_d[b, :, dc * 128 : (dc + 1) * 128],
                            )

                    return run

                def qk_unit(ft):
                    def run():
                        if "qkT" not in st:
                            st["qkT"] = qkp.tile(
                                [128, 12, NP], bf16, tag="qkT", name="qkT"
                            )
                        xT, qkT = st["xT"], st["qkT"]
                        pq = pmp.tile([128, 1024], f32, tag="pm")
                        for kc in range(6):
                            for c in range(2):
                                nc.tensor.matmul(
                                    pq[:, c * 512 : c * 512 + CH],
                                    wqkv[:, kc, ft * 128 : (ft + 1) * 128],
                                    xT[:, kc, c * CH : (c + 1) * CH],
                                    start=(kc == 0),
                                    stop=(kc == 5),
                                )
                        nc.vector.tensor_copy(
                            out=qkT[:, ft, :].rearrange("p (c i) -> p c i", c=2),
                            in_=pq[:, :].rearrange("p (c z) -> p c z", c=2)[
                                :, :, 0:CH
                            ],
                        )

                    return run

                def v_unit(tt, t0, tn):
                    def run():
                        if "vext" not in st:
                            st["vext"] = vp.tile(
                                [128, len(TT), 12, DH + 1], bf16, tag="vext",
                                name="vext",
                            )
                        xT, vext = st["xT"], st["vext"]
                        pv = pmp.tile([128, 1024], f32, tag="pm")
                        for kc in range(6):
                            nc.tensor.matmul(
                                pv[0:tn, 0:512],
                                xT[:, kc, t0 : t0 + tn],
                                wqkv[:, kc, 1536:2048],
                                start=(kc == 0),
                                stop=(kc == 5),
                            )
                            nc.tensor.matmul(
                                pv[0:tn, 512:768],
                                xT[:, kc, t0 : t0 + tn],
                                wqkv[:, kc, 2048:2304],
                                start=(kc == 0),
                                stop=(kc == 5),
                            )
                        nc.vector.tensor_copy(
                            out=vext[0:tn, tt, :, 0:DH],
                            in_=pv[0:tn, 0:768].rearrange("p (h d) -> p h d", h=12),
                        )
                        nc.vector.tensor_copy(
                            out=vext[0:tn, tt, :, DH : DH + 1],
                            in_=omaster[0:tn, :].rearrange("p (h o) -> p h o", o=1),
                        )

                    return run

                units.append(xt_unit())
                for ft in range(12):
                    units.append(qk_unit(ft))
                for tt, (t0, tn) in enumerate(TT):
                    units.append(v_unit(tt, t0, tn))
                return units

            def emit_head(b, h, attn_out):
                st = state[b]
                qkT, vext = st["qkT"], st["vext"]
                r0 = (h % 2) * 64
                qf = h // 2
                kf = 6 + h // 2
                a_tiles = []
                for jt, (j0, jn) in enumerate(TT):
                    pd = pdp.tile([128, 2, 512], f32, tag="pd")
                    for c in range(2):
                        nc.tensor.matmul(
                            pd[0:jn, c, 0:CH],
                            qkT[r0 : r0 + 64, kf, j0 : j0 + jn],
                            qkT[r0 : r0 + 64, qf, c * CH : (c + 1) * CH],
                            start=True,
                            stop=True,
                        )
                    A = apl.tile([128, NP], bf16, tag="A")
                    nc.scalar.activation(
                        out=A[0:jn, :].rearrange("p (c i) -> p c i", c=2),
                        in_=pd[0:jn, :, 0:CH],
                        func=AF.Exp,
                    )
                    nc.vector.tensor_tensor(
                        out=A[0:jn, j0 : j0 + jn],
                        in0=A[0:jn, j0 : j0 + jn],
                        in1=mask[0:jn, 0:jn],
                        op=OP.mult,
                    )
                    a_tiles.append(A)

                oe = pop.tile([128, 2, 512], f32, tag="oe")
                for jt, (j0, jn) in enumerate(TT):
                    A = a_tiles[jt]
                    for c in range(2):
                        nc.tensor.matmul(
                            oe[0:65, c, 0:CH],
                            vext[0:jn, jt, h, :],
                            A[0:jn, c * CH : (c + 1) * CH],
                            start=(jt == 0),
                            stop=(jt == len(TT) - 1),
                        )
                rsum = smp.tile([1, NP], f32, tag="rsum")
                nc.vector.tensor_copy(
                    out=rsum[:, :].rearrange("p (c i) -> p c i", c=2),
                    in_=oe[64:65, :, 0:CH],
                )
                recip = smp.tile([1, NP], f32, tag="recip")
                nc.vector.reciprocal_approx_fast(out=recip[:, :], in_=rsum[:, :])
                bcast = smp.tile([64, NP], f32, tag="bcast")
                nc.gpsimd.partition_broadcast(bcast[:, :], recip[0:1, :])
                nc.vector.tensor_tensor(
                    out=attn_out[r0 : r0 + 64, qf, :].rearrange(
                        "p (c i) -> p c i", c=2
                    ),
                    in0=oe[0:64, :, 0:CH],
                    in1=bcast[:, :].rearrange("p (c i) -> p c i", c=2),
                    op=OP.mult,
                )

            def outproj_units(b, attn_out):
                def unit(tt, t0, tn):
                    def run():
                        _outproj_tile(b, attn_out, tt, t0, tn)

                    return run

                return [unit(tt, t0, tn) for tt, (t0, tn) in enumerate(TT)]

            def _outproj_tile(b, attn_out, tt, t0, tn):
                if True:
                    po = pmp.tile([128, 1024], f32, tag="pm")
                    for kc in range(6):
                        nc.tensor.matmul(
                            po[0:tn, 0:512],
                            attn_out[:, kc, t0 : t0 + tn],
                            wout[:, kc, 0:512],
                            start=(kc == 0),
                            stop=(kc == 5),
                        )
                        nc.tensor.matmul(
                            po[0:tn, 512:768],
                            attn_out[:, kc, t0 : t0 + tn],
                            wout[:, kc, 512:768],
                            start=(kc == 0),
                            stop=(kc == 5),
                        )
                    ost = osp.tile([128, DIM], f32)
                    nc.vector.tensor_tensor(
                        out=ost[0:tn, :],
                        in0=po[0:tn, 0:768],
                        in1=b_bc[0:tn, :],
                        op=OP.add,
                    )
                    nc.sync.dma_start(
                        out=out_d[b, t0 : t0 + tn, :], in_=ost[0:tn, :]
                    )

            # ---- software pipeline over batches ----
            # attention(b) interleaves prep(b+1); all deferred out-projections
            # (batches 0..BPC-2) fill the final batch's attention stream.
            for u in prep_units(0):
                u()
            aouts = {}
            for b in range(BPC):
                units = prep_units(b + 1)
                if b == BPC - 1:
                    for pb in range(BPC - 1):
                        units = units + outproj_units(pb, aouts[pb])
                ui = 0
                aouts[b] = aop.tile(
                    [128, 6, NP], f32r, tag="attn_out", name="attn_out"
                )
                for h in range(12):
                    emit_head(b, h, aouts[b])
                    want = (len(units) * (h + 1)) // 12
                    while ui < want:
                        units[ui]()
                        ui += 1
            for u in outproj_units(BPC - 1, aouts[BPC - 1]):
                u()

    nc.compile()
    return nc


def _get_nc():
    if "nc" not in _cache:
        _cache["nc"] = _build_nc()
    return _cache["nc"]


def prepare_in_maps(inputs):
    import ml_dtypes

    bf = ml_dtypes.bfloat16
    x = np.asarray(inputs["x"], dtype=np.float32)
    W_qkv = np.asarray(inputs["W_qkv"], dtype=np.float32)
    scale = np.asarray(inputs["scale"], dtype=np.float32)
    W_out = np.ascontiguousarray(np.asarray(inputs["W_out"], dtype=np.float32))
    b_out = np.ascontiguousarray(np.asarray(inputs["b_out"], dtype=np.float32))

    # fold per-head LSA scale into the q columns of W_qkv
    Wq = W_qkv.copy()
    Wq[:, : H * DH] *= np.repeat(scale, DH)[None, :]
    Wq = np.ascontiguousarray(Wq.astype(bf))

    # pad tokens to XW with zeros, cast to bf16
    x_pad = np.zeros((B, XW, DIM), dtype=bf)
    x_pad[:, :N, :] = x.astype(bf)

    mask = np.ascontiguousarray((1.0 - np.eye(128, dtype=np.float32)).astype(bf))

    return [
        {
            "x": np.ascontiguousarray(x_pad[i * BPC : (i + 1) * BPC]),
            "wqkv": Wq,
            "wout": W_out,
            "bout": b_out,
            "mask": mask,
        }
        for i in range(NCORES)
    ]


def kernel(**inputs):
    from concourse import bass_utils

    nc = _get_nc()
    in_maps = prepare_in_maps(inputs)
    res = bass_utils.run_bass_kernel_spmd(nc, in_maps, core_ids=list(range(NCORES)))
    out = np.concatenate([res.results[i]["out"] for i in range(NCORES)], axis=0)
    return out.astype(np.float32)
